# revision 1
# baseline (speedup 1.0000x reference)
"""Trainium2 Bass kernel: fused QKV + RoPE + causal/windowed GQA attention
+ output projection.

Sharding: tensor-parallel by head across 8 cores. Core c owns q-heads
4c..4c+3 and KV-group c, plus the 512 w_o columns for those heads. Each
core computes a full-shape partial of the final output (contraction over
its 512 attention-output dims); the host sums the 8 partials. No device
collectives.

Performance structure (vs the plain-bf16 version of the same dataflow):
  - P1 (qkv proj) and P3 (out proj) run as scaled 3-term hi/lo fp8(e4m3)
    DoubleRow matmuls: A@B ~= Ah@Bh (two 128-contraction chunks per DR
    instruction) + (Ah@Bl + Al@Bh) (one DR instruction per chunk), for
    0.75x the bf16 PE cost at better-than-bf16 accuracy. Power-of-2
    pre-scales (x*16, w*1024, outT*32) keep the lo residuals out of fp8
    subnormals; the inverse scales fold into the existing evict /
    normalization constants and a final host-side divide.
  - softmax denominators come from 1-cycle column matmuls (lhsT = est
    q-slice, rhs = ones/32 -> out [q,1] accumulated in PSUM), replacing
    512-cycle row-sum matmuls; a DVE 32x32 block transpose + SBUF-to-SBUF
    DMA fold + one Pool partition_broadcast turn the reciprocal column
    into the [128,QG] normalization factor.
  - scores/PV stay bf16 (fp8 there fails the 2e-2 gate: single-fp8 q/k/
    est/v each contribute ~2.4-3.4e-2 error).
  - scheduling: engines are in-order, so consumption (denominator + PV)
    is staggered two chunks behind the score matmuls, P3 is software-
    pipelined one window-group behind P2 with its matmul groups laced
    into the chunk stream, batch-0 RoPE/v-transpose runs inside the P1
    scope, and batch-1 q-RoPE is prefetched one window-group ahead.
"""

import math
import sys
from contextlib import ExitStack

import numpy as np

sys.path.insert(0, "/opt/trn_rl_repo")

import ml_dtypes

BF16NP = ml_dtypes.bfloat16
F8NP = ml_dtypes.float8_e4m3

import concourse.bass as bass
import concourse.mybir as mybir
import concourse.tile as tile
from concourse import bacc

F32 = mybir.dt.float32
BF16 = mybir.dt.bfloat16
F8 = mybir.dt.float8e4
DR = mybir.MatmulPerfMode.DoubleRow

B, T, D = 2, 2048, 4096
H, G, HD = 32, 8, 128
THETA = 10000.0
NCORES = 8
HL = H // NCORES            # 4 local q heads
TOK = B * T                 # 4096
QROWS = HL * HD             # 512 local q rows
E = QROWS + 2 * HD          # 768 local qkv rows
SCALE = 1.0 / math.sqrt(HD)

SX = 16.0                   # x pre-scale (host)
SW = 1024.0                 # w_qkv / w_o pre-scale (host)
SO = 32.0                   # outT pre-scale (via ones=1/SO)
PSCALE = SX * SW            # P1 psum scale
P3SCALE = SO * SW           # P3 psum scale (divided out on host)

TOKG = 256                  # P1 token-group width
NTOKG = TOK // TOKG
NDC = D // 128              # 32 contraction chunks
NE = E // 128               # 6 qkv row chunks
QG = 512                    # P2 query-group width (within batch)
NQG = T // QG               # 4
NKC = T // 128              # 16 key chunks per batch
NSL = QG // 128             # 4 q-slices per window


def _mask_plan(window: int):
    """Per (qgroup, kchunk): 'skip', 'full', or a mask-key (delta-based)."""
    plan = {}
    keys = {}
    for g in range(NQG):
        for kc in range(NKC):
            i_min, i_max = QG * g, QG * g + QG - 1
            j_min, j_max = 128 * kc, 128 * kc + 127
            if j_min > i_max or (i_min - j_max) >= window:
                plan[(g, kc)] = ("skip", None)
            elif j_max <= i_min and (i_max - j_min) < window:
                plan[(g, kc)] = ("full", None)
            else:
                key = QG * g - 128 * kc
                if key not in keys:
                    keys[key] = len(keys)
                plan[(g, kc)] = ("mask", keys[key])
    return plan, keys


def _build_masks(window: int, keys: dict) -> np.ndarray:
    n = max(1, len(keys))
    m = np.zeros((n, 128, QG), dtype=np.float32)
    for key, idx in keys.items():
        qq = np.arange(QG)[None, :]
        kk = np.arange(128)[:, None]
        diff = key + qq - kk          # i - j
        vis = (diff >= 0) & (diff < window)
        m[idx] = np.where(vis, 1.0, 0.0)
    return m


PAIRSWAP = [i ^ 1 for i in range(32)]


def _rope_ops(nc, pool, dst, src, cos_ap, sin_ap):
    """Interleaved-pair RoPE: dst = src*cos + pairswap(src)*signed_sin."""
    W = dst.shape[-1]
    sw = pool.tile([128, W], BF16, tag="rope_sw")
    tmp = pool.tile([128, W], BF16, tag="rope_tmp")
    qc = pool.tile([128, W], BF16, tag="rope_qc")
    mult = mybir.AluOpType.mult
    nc.vector.stream_shuffle(sw, src, PAIRSWAP)
    nc.vector.tensor_tensor(tmp, sw, sin_ap, mult)
    nc.vector.tensor_tensor(qc, src, cos_ap, mult)
    nc.vector.tensor_tensor(dst, qc, tmp, mybir.AluOpType.add)


def build_nc(window: int):
    plan, keys = _mask_plan(window)
    nmask = max(1, len(keys))

    nc = bacc.Bacc()
    xhl_d = nc.dram_tensor("xhl", [NTOKG, 128, NDC, 2, TOKG], F8, kind="ExternalInput")
    whl_d = nc.dram_tensor("whl", [128, NDC, 2, E], F8, kind="ExternalInput")
    wohl_d = nc.dram_tensor("wohl", [128, HL, 2, D], F8, kind="ExternalInput")
    cos_d = nc.dram_tensor("cosH", [128, T], BF16, kind="ExternalInput")
    sin_d = nc.dram_tensor("sinH", [128, T], BF16, kind="ExternalInput")
    masks_d = nc.dram_tensor("masks", [nmask, 128, QG], BF16, kind="ExternalInput")
    ident_d = nc.dram_tensor("ident", [128, 128], BF16, kind="ExternalInput")
    out_d = nc.dram_tensor("out", [TOK, D], BF16, kind="ExternalOutput")

    # window-dependent mask storage eats SBUF: shed pipeline depth
    est_bufs = 8 if nmask <= 4 else 4
    sc0_bufs = 2 if nmask <= 4 else 1
    pan_bufs = 3 if nmask <= 4 else 2

    with ExitStack() as octx:
        tc = octx.enter_context(tile.TileContext(nc))
        qkvp = octx.enter_context(tc.tile_pool(name="qkvT", bufs=1))
        # per-batch tiles so batch-0 RoPE/attention can start while P1 still
        # computes batch 1 (dependency tracking is tile-granular)
        qkvT_sb = [[qkvp.tile([128, T], BF16, tag=f"qkv{e}b{b}",
                              name=f"qkv{e}b{b}") for b in range(B)]
                   for e in range(NE)]

        # kv-phase constants allocated up front so RoPE / v-transpose can be
        # emitted inside the P1 scope (overlapping P1's tail)
        p2ctx = ExitStack()
        kpool = p2ctx.enter_context(tc.tile_pool(name="kv", bufs=1))
        # outT slots (hi, lo); wo slots (lo, hi)
        outT = kpool.tile([128, HL, 2, TOK], F8, name="outThl")
        vsb = kpool.tile([128, TOK // 128, 128], BF16, tag="v")
        cos_sb = kpool.tile([128, T], BF16, tag="cos")
        sin_sb = kpool.tile([128, T], BF16, tag="sin")
        ones_sb = kpool.tile([128, 1], BF16, tag="ones")
        mask_sb = kpool.tile([128, nmask, QG], BF16, tag="masks")
        ident = kpool.tile([128, 128], BF16, tag="ident")

        nc.vector.memset(ones_sb, 1.0 / SO)

        # ---------------- P1: qkvT = w^T @ xT  (3-term fp8 DR) -------------
        with ExitStack() as ctx:
            wpool = ctx.enter_context(tc.tile_pool(name="w1", bufs=1))
            xpool = ctx.enter_context(tc.tile_pool(name="x1", bufs=2))
            ppool = ctx.enter_context(tc.tile_pool(name="ps1", bufs=6, space="PSUM"))
            sc0 = ctx.enter_context(tc.tile_pool(name="p2a", bufs=sc0_bufs))
            pt0 = ctx.enter_context(tc.tile_pool(name="p2aps", bufs=2, space="PSUM"))

            def emit_batch_prep(b):
                """v transpose (+ in-place RoPE for batch 0 only; batch-1
                rope is spread through the batch-0 window stream)."""
                vT = qkvT_sb[HL + 1][b]
                for tcl in range(T // 128):
                    pst = pt0.tile([128, 128], BF16, tag="tr")
                    nc.tensor.transpose(
                        pst, vT[:, tcl * 128:(tcl + 1) * 128], ident)
                    if b == 0 or tcl % 2 == 0:
                        nc.scalar.copy(vsb[:, b * NKC + tcl, :], pst)
                    else:
                        nc.vector.tensor_copy(vsb[:, b * NKC + tcl, :], pst)
                if b == 0:
                    kslice = qkvT_sb[HL][b][:]
                    _rope_ops(nc, sc0, kslice, kslice, cos_sb, sin_sb)
                    for hh in range(HL):
                        qslice = qkvT_sb[hh][b][:]
                        _rope_ops(nc, sc0, qslice, qslice, cos_sb, sin_sb)

            # w slots (hi, lo); x slots (lo, hi) -> cross instr pairs
            # wh*xl + wl*xh naturally; main uses w slot0 x slot1.
            wsb = wpool.tile([128, NDC, 2, E], F8)
            # fine-grained alternating w-hi / first-x pieces, then w-lo
            # pieces: PE consumes each 4-dc piece as it lands
            xsb0 = xpool.tile([128, NDC, 2, TOKG], F8, tag="xslab",
                              name="xslab0")
            for dq in range(8):
                dsl = slice(dq * 4, (dq + 1) * 4)
                nc.sync.dma_start(out=wsb[:, dsl, 0, :],
                                  in_=whl_d[:, dsl, 0, :])
                nc.sync.dma_start(out=xsb0[:, dsl],
                                  in_=xhl_d[0, :, dsl])
            for dq in range(8):
                dsl = slice(dq * 4, (dq + 1) * 4)
                nc.sync.dma_start(out=wsb[:, dsl, 1, :],
                                  in_=whl_d[:, dsl, 1, :])

            for gi in range(NTOKG):
                if gi == 0:
                    xsb = xsb0
                else:
                    xsb = xpool.tile([128, NDC, 2, TOKG], F8, tag="xslab")
                    for dq in range(4):
                        nc.sync.dma_start(out=xsb[:, dq * 8:(dq + 1) * 8],
                                          in_=xhl_d[gi, :, dq * 8:(dq + 1) * 8])
                if gi == 2:
                    # constants ride behind the startup-critical loads; they
                    # are first needed by the rope/transpose prep much later
                    nc.sync.dma_start(out=cos_sb, in_=cos_d[:])
                    nc.sync.dma_start(out=sin_sb, in_=sin_d[:])
                    nc.sync.dma_start(
                        out=mask_sb,
                        in_=masks_d[:].rearrange("n p q -> p n q"))
                    nc.sync.dma_start(out=ident, in_=ident_d[:])

                def emit_main(e, ps):
                    ecols = slice(e * 128, (e + 1) * 128)
                    for kp in range(NDC // 2):
                        nc.tensor.matmul(
                            ps,
                            lhsT=wsb[:, 2 * kp:2 * kp + 2, 0, ecols],
                            rhs=xsb[:, 2 * kp:2 * kp + 2, 1, :],
                            start=(kp == 0), stop=False, perf_mode=DR)

                def emit_cross(e, ps):
                    ecols = slice(e * 128, (e + 1) * 128)
                    for k in range(NDC):
                        nc.tensor.matmul(
                            ps,
                            lhsT=wsb[:, k, :, ecols],
                            rhs=xsb[:, k, :, :],
                            start=False, stop=(k == NDC - 1), perf_mode=DR)
                    gb = gi // (NTOKG // B)
                    gcol = (gi % (NTOKG // B)) * TOKG
                    nc.scalar.mul(
                        qkvT_sb[e][gb][:, gcol:gcol + TOKG], ps,
                        (SCALE / PSCALE) if e < HL else (1.0 / PSCALE))

                if gi == 0:
                    # kp-major then k-major: consume DMA pieces as they land
                    pss = [ppool.tile([128, TOKG], F32, tag="p1",
                                      name=f"p1z{e}") for e in range(NE)]
                    for kp in range(NDC // 2):
                        for e in range(NE):
                            ecols = slice(e * 128, (e + 1) * 128)
                            nc.tensor.matmul(
                                pss[e],
                                lhsT=wsb[:, 2 * kp:2 * kp + 2, 0, ecols],
                                rhs=xsb[:, 2 * kp:2 * kp + 2, 1, :],
                                start=(kp == 0), stop=False, perf_mode=DR)
                    for k in range(NDC):
                        for e in range(NE):
                            ecols = slice(e * 128, (e + 1) * 128)
                            nc.tensor.matmul(
                                pss[e],
                                lhsT=wsb[:, k, :, ecols],
                                rhs=xsb[:, k, :, :],
                                start=False, stop=(k == NDC - 1),
                                perf_mode=DR)
                    for e in range(NE):
                        nc.scalar.mul(
                            qkvT_sb[e][0][:, 0:TOKG], pss[e],
                            (SCALE / PSCALE) if e < HL else (1.0 / PSCALE))
                else:
                    for e in range(NE):
                        ps = ppool.tile([128, TOKG], F32, tag="p1")
                        emit_main(e, ps)
                        emit_cross(e, ps)
                if gi == NTOKG // B + 1:
                    emit_batch_prep(0)  # batch-0 prep overlaps P1 batch 1
            emit_batch_prep(1)

        # ---------------- P2 + interleaved P3 ----------------
        with ExitStack() as ctx:
            spool = ctx.enter_context(tc.tile_pool(name="sc2", bufs=2))
            estp = ctx.enter_context(tc.tile_pool(name="est", bufs=est_bufs))
            wpool = ctx.enter_context(tc.tile_pool(name="wo", bufs=1))
            panp = ctx.enter_context(tc.tile_pool(name="pan", bufs=pan_bufs))
            stps = ctx.enter_context(tc.tile_pool(name="stps", bufs=2, space="PSUM"))
            dnps = ctx.enter_context(tc.tile_pool(name="dnps", bufs=1, space="PSUM"))
            ops = ctx.enter_context(tc.tile_pool(name="ops", bufs=2, space="PSUM"))
            pps = ctx.enter_context(tc.tile_pool(name="ps3", bufs=3, space="PSUM"))

            wo_sb = wpool.tile([128, HL, 2, D], F8, name="wohl")
            for ch in range(HL):
                nc.sync.dma_start(out=wo_sb[:, ch], in_=wohl_d[:, ch])

            # P3 is software-pipelined one (b,g) behind P2: its matmul groups
            # are emitted interleaved into the NEXT window-group's chunk
            # stream, filling the PE gaps where the in-order PE queue would
            # otherwise stall waiting on ACT's exp.
            p3_pending = []
            panel_state = {}
            cur_hh = [3]

            def make_p3(b, g):
                groups = []
                for tloc in range(QG // 128):
                    tch = (b * T + g * QG) // 128 + tloc
                    tcols = slice(tch * 128, (tch + 1) * 128)
                    for et in range(D // 512):
                        groups.append((tch, tcols, et))
                return groups

            def emit_p3_group(grp):
                tch, tcols, et = grp
                if et == 0:
                    panel_state[tch] = panp.tile([128, D], BF16, tag="panel",
                                                 name=f"panel{tch}")
                panel = panel_state[tch]
                ps = pps.tile([128, 512], F32, tag="p3")
                dcols = slice(et * 512, (et + 1) * 512)
                for hp in range(HL // 2):
                    nc.tensor.matmul(
                        ps,
                        lhsT=outT[:, 2 * hp:2 * hp + 2, 0, tcols],
                        rhs=wo_sb[:, 2 * hp:2 * hp + 2, 1, dcols],
                        start=(hp == 0), stop=False, perf_mode=DR)
                for h in range(HL):
                    nc.tensor.matmul(
                        ps,
                        lhsT=outT[:, h, :, tcols],
                        rhs=wo_sb[:, h, :, dcols],
                        start=False, stop=(h == HL - 1), perf_mode=DR)
                # Pool cannot read PSUM; keep ACT clean while the exp
                # pipeline ramps (hh 0/1), else alternate DVE/ACT
                dcl = slice(et * 512, (et + 1) * 512)
                if cur_hh[0] < 2 or (tch * 8 + et) % 2 == 0:
                    nc.vector.tensor_copy(panel[:, dcl], ps)
                else:
                    nc.scalar.copy(panel[:, dcl], ps)
                if et == D // 512 - 1:
                    nc.sync.dma_start(
                        out=out_d[tch * 128:(tch + 1) * 128, :], in_=panel)
                    del panel_state[tch]

            qsb_ring = {}

            def prefetch_b1_rope(g, hh):
                csl = slice(g * QG, (g + 1) * QG)
                qsb = spool.tile([128, QG], BF16, tag=f"qsb{hh}",
                                 name=f"qsb{hh}")
                _rope_ops(nc, spool, qsb, qkvT_sb[hh][1][:, csl],
                          cos_sb[:, csl], sin_sb[:, csl])
                qsb_ring[(g, hh)] = qsb

            for b in range(B):
                for g in range(NQG):
                    if b == 0 and g == 2:
                        # rope batch-1 k rows here: DVE has slack while the
                        # batch-0 windows run
                        kslice = qkvT_sb[HL][1][:]
                        _rope_ops(nc, spool, kslice, kslice, cos_sb, sin_sb)
                    if b == 0 and g == 3:
                        for hh in range(HL):
                            prefetch_b1_rope(0, hh)
                    for hh in range(HL):
                        cur_hh[0] = hh
                        csl = slice(g * QG, (g + 1) * QG)
                        if b == 0:
                            q_ap = qkvT_sb[hh][b][:, csl]
                        else:
                            q_ap = qsb_ring.pop((g, hh))
                            if g < NQG - 1:
                                prefetch_b1_rope(g + 1, hh)
                        vis = [(kc, plan[(g, kc)]) for kc in range(NKC)
                               if plan[(g, kc)][0] != "skip"]
                        bounds = []
                        for kc, (kind, mid) in vis:
                            aoff = QG * g - 128 * kc
                            qlo = max(0, -aoff)
                            qhi = min(QG, window - aoff + 127)
                            qhi = min(QG, -(-qhi // 128) * 128)  # align up
                            bounds.append((qlo, qhi))
                        tot_dn = sum((qh - ql) // 128 for ql, qh in bounds)
                        dn = dnps.tile([128, NSL], F32, tag="dn")
                        o_ps = ops.tile([128, QG], F32, tag="o")
                        ndn = 0
                        ests = {}

                        def emit_score(idx):
                            kc, (kind, mid) = vis[idx]
                            qlo, qhi = bounds[idx]
                            qsl = slice(qlo, qhi)
                            st = stps.tile([128, QG], F32, tag="st")
                            nc.tensor.matmul(
                                st[:, qsl],
                                lhsT=qkvT_sb[HL][b][:, kc * 128:
                                                    (kc + 1) * 128],
                                rhs=q_ap[:, qsl],
                                start=True, stop=True)
                            est = estp.tile([128, QG], BF16, tag="est")
                            nc.scalar.activation(
                                est[:, qsl], st[:, qsl],
                                mybir.ActivationFunctionType.Exp)
                            if kind == "mask":
                                nc.vector.tensor_tensor(
                                    est[:, qsl], est[:, qsl],
                                    mask_sb[:, mid, qsl],
                                    mybir.AluOpType.mult)
                            ests[idx] = est

                        def emit_consume(idx):
                            nonlocal ndn
                            kc, _ = vis[idx]
                            qlo, qhi = bounds[idx]
                            qsl = slice(qlo, qhi)
                            est = ests.pop(idx)
                            # denominator columns: out [q,1], 1 cycle each
                            for j in range(qlo // 128, qhi // 128):
                                nc.tensor.matmul(
                                    dn[:, j:j + 1],
                                    lhsT=est[:, j * 128:(j + 1) * 128],
                                    rhs=ones_sb,
                                    start=(ndn == 0), stop=(ndn == tot_dn - 1))
                                ndn += 1
                            nc.tensor.matmul(
                                o_ps[:, qsl],
                                lhsT=vsb[:, b * NKC + kc, :],
                                rhs=est[:, qsl],
                                start=(idx == 0), stop=(idx == len(vis) - 1))

                        # stagger consumption two chunks behind scores;
                        # lace one pending P3 group between chunks (not in
                        # hh=0: the previous group's outT may not be final)
                        lag = 2
                        can_pop = hh > 0
                        for idx in range(len(vis)):
                            emit_score(idx)
                            if idx >= lag:
                                emit_consume(idx - lag)
                                if can_pop and p3_pending:
                                    _, grp = p3_pending.pop(0)
                                    emit_p3_group(grp)
                        for idx in range(max(0, len(vis) - lag), len(vis)):
                            emit_consume(idx)
                            if can_pop and p3_pending:
                                _, grp = p3_pending.pop(0)
                                emit_p3_group(grp)

                        # 1/denom -> row layout -> broadcast
                        dnr = spool.tile([128, 32], F32, tag="dnr")
                        nc.vector.memset(dnr[:, NSL:32], 1.0)
                        nc.vector.reciprocal(dnr[:, 0:NSL], dn)
                        trev = spool.tile([32, 128], F32, tag="trev")
                        for tb in range(4):
                            nc.vector.transpose(
                                trev[0:32, tb * 32:(tb + 1) * 32],
                                dnr[tb * 32:(tb + 1) * 32, 0:32])
                        # engine APs must be quadrant-aligned, so fold
                        # the 4 slice-rows into one partition-0 row via DMA,
                        # then broadcast once
                        r0 = spool.tile([1, QG], F32, tag="r0")
                        nc.sync.dma_start(
                            out=r0[:].rearrange("o (s q) -> o s q", s=NSL),
                            in_=trev[0:NSL, :])
                        rb = spool.tile([128, QG], F32, tag="rb")
                        nc.gpsimd.partition_broadcast(rb, r0)
                        tmp = spool.tile([128, QG], BF16, tag="otmp")
                        nc.vector.tensor_tensor(
                            tmp, o_ps, rb, mybir.AluOpType.mult)
                        wsl = slice(b * T + g * QG, b * T + (g + 1) * QG)
                        nc.vector.tensor_copy(outT[:, hh, 0, wsl], tmp)
                        nc.vector.tensor_tensor(
                            outT[:, hh, 1, wsl], tmp, outT[:, hh, 0, wsl],
                            mybir.AluOpType.subtract)

                    bgidx = b * NQG + g
                    p3_pending.extend(
                        (bgidx, grp) for grp in make_p3(b, g))
            for _, grp in p3_pending:
                emit_p3_group(grp)

        p2ctx.close()

    nc.finalize()
    return nc, nmask


_CACHE = {}


def _get_nc(window: int):
    if window not in _CACHE:
        _CACHE[window] = build_nc(window)
    return _CACHE[window]


LAST_RESULTS = None


def _hilo(a32, scale):
    """fp8 hi/lo split of a32*scale; returns (hi, lo) as float8_e4m3."""
    s = (a32 * scale).astype(np.float32)
    hi = s.astype(F8NP)
    lo = (s - hi.astype(np.float32)).astype(F8NP)
    return hi, lo


def kernel(x, w_qkv, w_o, window_size, _trace=False):
    window = int(window_size)
    nc, nmask = _get_nc(window)
    _, keys = _mask_plan(window)
    masks = _build_masks(window, keys)

    xT = np.ascontiguousarray(x.reshape(TOK, D).T)
    xh, xl = _hilo(xT, SX)
    # [NTOKG, 128, NDC, 2, TOKG] slots (lo, hi): token-group-major so
    # each DMA descriptor is a contiguous 16KB per-partition run
    xhl = np.ascontiguousarray(
        np.stack([xl, xh], axis=1)
        .reshape(NDC, 128, 2, NTOKG, TOKG)
        .transpose(3, 1, 0, 2, 4))

    inv = 1.0 / (THETA ** (np.arange(0, HD, 2, dtype=np.float64) / HD))
    freqs = np.arange(T, dtype=np.float64)[:, None] * inv[None, :]  # [T, 64]
    cosH = np.repeat(np.cos(freqs).T, 2, axis=0).astype(BF16NP)  # [128, T]
    sign = np.where(np.arange(HD) % 2 == 0, -1.0, 1.0)[:, None]
    sinH = (np.repeat(np.sin(freqs).T, 2, axis=0) * sign).astype(BF16NP)
    ident = np.eye(128).astype(BF16NP)

    in_maps = []
    for c in range(NCORES):
        wq = w_qkv[QROWS * c:QROWS * (c + 1)]
        wk = w_qkv[H * HD + HD * c: H * HD + HD * (c + 1)]
        wv = w_qkv[H * HD + G * HD + HD * c: H * HD + G * HD + HD * (c + 1)]
        wqkvT = np.ascontiguousarray(
            np.concatenate([wq, wk, wv], axis=0).T)  # [D, E] f32
        wh, wl = _hilo(wqkvT, SW)
        whl = np.ascontiguousarray(
            np.stack([wh, wl], axis=1).reshape(NDC, 128, 2, E)
            .transpose(1, 0, 2, 3))  # [128, NDC, 2, E] slots (hi, lo)
        woT = np.ascontiguousarray(
            w_o[:, QROWS * c:QROWS * (c + 1)].T)  # [512, D] f32
        woh, wol = _hilo(woT, SW)
        wohl = np.ascontiguousarray(
            np.stack([wol, woh], axis=1).reshape(HL, 128, 2, D)
            .transpose(1, 0, 2, 3))  # [128, HL, 2, D] slots (lo, hi)
        in_maps.append({
            "xhl": xhl, "whl": whl, "wohl": wohl,
            "cosH": cosH, "sinH": sinH, "masks": masks.astype(BF16NP),
            "ident": ident,
        })

    from concourse.bass_utils import run_bass_kernel_spmd
    res = run_bass_kernel_spmd(nc, in_maps, core_ids=list(range(NCORES)),
                               trace=_trace)
    global LAST_RESULTS
    LAST_RESULTS = res
    acc = res.results[0]["out"].astype(np.float32).copy()
    for c in range(1, NCORES):
        acc += res.results[c]["out"]
    acc /= P3SCALE
    return acc.reshape(B, T, D)

